# revision 1
# baseline (speedup 1.0000x reference)
"""LightGCN (3-layer propagation + BCE loss) on 8 Trainium2 NeuronCores.

Strategy (row-sharded graph parallelism):
  - Rows (segment-sum targets) are sharded across the 8 cores; edges are
    sorted by destination row on the host and bucketed into "groups" of
    <=128 consecutive rows whose edges are padded to G*128 slots.
  - Per group the device: indirect-gathers the 14*128 source embeddings,
    scales by edge_val (batched DVE op), builds a one-hot selection matrix
    via is_equal against an iota tile (batched DVE op), and accumulates
    sel.T @ msg into PSUM with 14 chained matmuls -> the segment sums for
    the group's rows.  Result rows are indirect-scattered to the core's
    local shard of the next-layer table.
  - An AllGather per layer rebuilds the full [N,64] table for the next
    layer's gathers.  Layer-mean pooling is accumulated per-shard in fp32.
  - The final BCE-with-logits loss is computed redundantly on every core
    from the AllGathered mean table; core 0's scalar is returned.
"""

import sys

sys.path.insert(0, "/opt/trn_rl_repo")

import numpy as np

import concourse.bacc as bacc
import concourse.bass as bass
import concourse.mybir as mybir
import concourse.tile as tile
from concourse import bass_utils

F16 = mybir.dt.float16
F32 = mybir.dt.float32
I32 = mybir.dt.int32

# ---- problem constants (hardcoded; kernel.py must be self-contained) ----
REAL = dict(
    n_users=100001,  # user rows in the table (NUM_USERS + 1)
    n_total=150001,
    d=64,
    n_layers=3,
    batch=8192,
    n_blocks=147,  # 147*128 = 18816 rows per core
    G=14,  # 128-edge chunks per group
)
W = 8  # cores
P = 128


# ======================= host-side preprocessing ========================


def _build_core_tables(rows, cols, vals, r_lo, r_hi, cfg):
    """rows/cols/vals: this core's edges, rows sorted ascending (global ids).
    Returns (colpad [NG,S], valpad, rlpad, tgt [NG,128], NG)."""
    G = cfg["G"]
    S = G * P
    rcore = cfg["n_blocks"] * P
    rl = rows - r_lo  # local row ids, [0, rcore)
    counts = np.bincount(rl, minlength=rcore)
    # greedy grouping: consecutive rows, <=128 rows and <=S edges per group
    r0s, nrs, e0s, nes = [], [], [], []
    r, e = 0, 0
    while r < rcore:
        r0, er = r, 0
        while r < rcore and r - r0 < P and er + counts[r] <= S:
            er += counts[r]
            r += 1
        r0s.append(r0)
        nrs.append(r - r0)
        e0s.append(e)
        nes.append(er)
        e += er
    assert e == len(rl)
    NG = len(r0s)
    r0s = np.asarray(r0s)
    nrs = np.asarray(nrs)
    e0s = np.asarray(e0s)
    nes = np.asarray(nes)

    colpad = np.zeros((NG, S), np.int32)
    valpad = np.zeros((NG, S), np.float16)
    rlpad = np.zeros((NG, S), np.float16)
    gi = np.repeat(np.arange(NG), nes)
    off = np.arange(len(rl)) - np.repeat(e0s, nes)
    dest = gi * S + off
    colpad.flat[dest] = cols
    valpad.flat[dest] = vals.astype(np.float16)
    rlpad.flat[dest] = (rl - np.repeat(r0s, nes)).astype(np.float16)

    pp = np.arange(P)
    tgt = np.where(pp[None, :] < nrs[:, None], r0s[:, None] + pp[None, :],
                   rcore + pp[None, :]).astype(np.int32)
    return colpad, valpad, rlpad, tgt, NG


def preprocess(users, items, labels, edge_row, edge_col, edge_val,
               user_emb, item_emb, cfg):
    """Build the 8 per-core input maps."""
    G = cfg["G"]
    S = G * P
    rcore = cfg["n_blocks"] * P
    n_pad = rcore * W
    d = cfg["d"]
    n_total = cfg["n_total"]

    order = np.argsort(edge_row, kind="stable")
    rows = edge_row[order]
    cols = edge_col[order]
    vals = edge_val[order]

    per_core = []
    bounds = np.searchsorted(rows, np.arange(W + 1) * rcore)
    for k in range(W):
        s, e = bounds[k], bounds[k + 1]
        per_core.append(
            _build_core_tables(rows[s:e], cols[s:e], vals[s:e],
                               k * rcore, (k + 1) * rcore, cfg))
    NG = max(pc[4] for pc in per_core)

    # full initial table, fp16, padded to n_pad rows
    table0 = np.zeros((n_pad, d), np.float16)
    table0[:user_emb.shape[0]] = user_emb.astype(np.float16)
    table0[user_emb.shape[0]:user_emb.shape[0] + item_emb.shape[0]] = \
        item_emb.astype(np.float16)
    # fp32 copy for the layer-mean accumulator
    t0f = np.zeros((n_pad, d), np.float32)
    t0f[:user_emb.shape[0]] = user_emb
    t0f[user_emb.shape[0]:user_emb.shape[0] + item_emb.shape[0]] = item_emb

    # iota pattern [128, G*128]: iota[p, c*128+r] = r
    iota = np.tile(np.arange(P, dtype=np.float16)[None, :], (P, G)).reshape(
        P, G * P)
    iota = np.ascontiguousarray(
        np.broadcast_to(np.arange(P, dtype=np.float16)[None, None, :],
                        (P, G, P)).reshape(P, G * P))
    ones = np.ones((P, 1), np.float32)

    bch = cfg["batch"] // P
    urow = users.astype(np.int32).reshape(P, bch)
    vrow = (cfg["n_users"] + items).astype(np.int32).reshape(P, bch)
    lab = labels.astype(np.float32).reshape(P, bch)

    in_maps = []
    for k in range(W):
        colpad, valpad, rlpad, tgt, ngk = per_core[k]

        def dev_layout(a, dtype):
            out = np.zeros((NG, S), dtype)
            out[:a.shape[0]] = a
            return np.ascontiguousarray(
                out.reshape(NG, G, P).transpose(2, 0, 1).reshape(P, NG * G))

        tgt_full = np.tile(rcore + np.arange(P, dtype=np.int32)[None, :],
                           (NG, 1))
        tgt_full[:ngk] = tgt
        t0loc = np.zeros((rcore + P, d), np.float32)
        t0loc[:rcore] = t0f[k * rcore:(k + 1) * rcore]
        in_maps.append({
            "table0": table0,
            "t0loc": t0loc,
            "colidx": dev_layout(colpad, np.int32),
            "val": dev_layout(valpad, np.float16),
            "rl": dev_layout(rlpad, np.float16),
            "tgt": np.ascontiguousarray(tgt_full.T),
            "iota": iota,
            "ones": ones,
            "urow": urow,
            "vrow": vrow,
            "labels": lab,
        })
    return in_maps, NG


# =========================== device program =============================


def build_program(cfg, NG):
    G = cfg["G"]
    d = cfg["d"]
    rcore = cfg["n_blocks"] * P
    n_pad = rcore * W
    lrows = rcore + P
    bch = cfg["batch"] // P
    n_layers = cfg["n_layers"]

    nc = bacc.Bacc("TRN2", target_bir_lowering=False, debug=False,
                   enable_asserts=False, num_devices=W)

    table0 = nc.dram_tensor("table0", [n_pad, d], F16, kind="ExternalInput")
    t0loc = nc.dram_tensor("t0loc", [lrows, d], F32, kind="ExternalInput")
    colidx = nc.dram_tensor("colidx", [P, NG * G], I32, kind="ExternalInput")
    val = nc.dram_tensor("val", [P, NG * G], F16, kind="ExternalInput")
    rl = nc.dram_tensor("rl", [P, NG * G], F16, kind="ExternalInput")
    tgt = nc.dram_tensor("tgt", [P, NG], I32, kind="ExternalInput")
    iota = nc.dram_tensor("iota", [P, G * P], F16, kind="ExternalInput")
    ones = nc.dram_tensor("ones", [P, 1], F32, kind="ExternalInput")
    urow = nc.dram_tensor("urow", [P, bch], I32, kind="ExternalInput")
    vrow = nc.dram_tensor("vrow", [P, bch], I32, kind="ExternalInput")
    labels = nc.dram_tensor("labels", [P, bch], F32, kind="ExternalInput")
    loss = nc.dram_tensor("loss", [1, 1], F32, kind="ExternalOutput")

    rg = [list(range(W))]
    AT = mybir.ActivationFunctionType

    with tile.TileContext(nc) as tc:
        with (
            tc.tile_pool(name="dram", bufs=1, space="DRAM") as dpool,
            tc.tile_pool(name="const", bufs=1) as cpool,
            tc.tile_pool(name="work", bufs=4) as wpool,
            tc.tile_pool(name="bulk", bufs=3) as bpool,
            tc.tile_pool(name="fin", bufs=1) as fpool,
            tc.tile_pool(name="psum", bufs=6, space="PSUM") as ppool,
        ):
            L = dpool.tile([lrows, d], F16)
            tabs = [dpool.tile([n_pad, d], F16, name=f"tab{i}",
                               addr_space="Shared")
                    for i in range(n_layers - 1)]
            M = dpool.tile([lrows, d], F32)
            Mfull = dpool.tile([n_pad, d], F32, addr_space="Shared")

            # static SBUF loads (layer-invariant)
            colidx_sb = cpool.tile([P, NG * G], I32)
            val_sb = cpool.tile([P, NG * G], F16)
            rl_sb = cpool.tile([P, NG * G], F16)
            tgt_sb = cpool.tile([P, NG], I32)
            iota_sb = cpool.tile([P, G * P], F16)
            ones_sb = cpool.tile([P, 1], F32)
            urow_sb = cpool.tile([P, bch], I32)
            vrow_sb = cpool.tile([P, bch], I32)
            lab_sb = cpool.tile([P, bch], F32)
            for sb, dr in ((colidx_sb, colidx), (val_sb, val), (rl_sb, rl),
                           (tgt_sb, tgt), (iota_sb, iota), (ones_sb, ones),
                           (urow_sb, urow), (vrow_sb, vrow), (lab_sb, labels)):
                nc.sync.dma_start(out=sb[:], in_=dr.ap())

            for ell in range(n_layers):
                tab = table0.ap() if ell == 0 else tabs[ell - 1][:, :]
                for g in range(NG):
                    gt = wpool.tile([P, G * d], F16, tag="gt")
                    nc.gpsimd.indirect_dma_start(
                        out=gt[:], out_offset=None, in_=tab,
                        in_offset=bass.IndirectOffsetOnAxis(
                            ap=colidx_sb[:, g * G:(g + 1) * G], axis=0))
                    msg = wpool.tile([P, G * d], F16, tag="msg")
                    nc.vector.tensor_tensor(
                        out=msg[:].rearrange("p (g d) -> p g d", g=G),
                        in0=gt[:].rearrange("p (g d) -> p g d", g=G),
                        in1=val_sb[:, g * G:(g + 1) * G].to_broadcast(
                            [P, G, d]),
                        op=mybir.AluOpType.mult)
                    sel = wpool.tile([P, G * P], F16, tag="sel")
                    nc.vector.tensor_tensor(
                        out=sel[:].rearrange("p (g r) -> p g r", g=G),
                        in0=rl_sb[:, g * G:(g + 1) * G].to_broadcast(
                            [P, G, P]),
                        in1=iota_sb[:].rearrange("p (g r) -> p g r", g=G),
                        op=mybir.AluOpType.is_equal)
                    ps = ppool.tile([P, d], F32)
                    for c in range(G):
                        nc.tensor.matmul(
                            out=ps[:], lhsT=sel[:, c * P:(c + 1) * P],
                            rhs=msg[:, c * d:(c + 1) * d],
                            start=(c == 0), stop=(c == G - 1))
                    ob = wpool.tile([P, d], F16, tag="ob")
                    nc.scalar.copy(out=ob[:], in_=ps[:])
                    nc.gpsimd.indirect_dma_start(
                        out=L[:, :],
                        out_offset=bass.IndirectOffsetOnAxis(
                            ap=tgt_sb[:, g:g + 1], axis=0),
                        in_=ob[:], in_offset=None)

                # M accumulation over the real rows (rows 0..rcore-1)
                n_in = cfg["n_blocks"]  # 147 columns of 128-row stripes
                nch = 3 if n_in % 3 == 0 else 1
                ch = n_in // nch
                L3 = L[0:rcore, :].rearrange("(p n) d -> p n d", p=P)
                M3 = M[0:rcore, :].rearrange("(p n) d -> p n d", p=P)
                S3 = (t0loc.ap() if ell == 0 else M[0:rcore, :]).rearrange(
                    "(p n) d -> p n d", p=P)
                for i in range(nch):
                    sl = slice(i * ch, (i + 1) * ch)
                    lt = bpool.tile([P, ch * d], F16, tag="lt")
                    nc.sync.dma_start(
                        out=lt[:].rearrange("p (n d) -> p n d", d=d),
                        in_=L3[:, sl, :])
                    ltf = bpool.tile([P, ch * d], F32, tag="ltf")
                    nc.scalar.copy(out=ltf[:], in_=lt[:])
                    mt = bpool.tile([P, ch * d], F32, tag="mt")
                    nc.sync.dma_start(
                        out=mt[:].rearrange("p (n d) -> p n d", d=d),
                        in_=S3[:, sl, :])
                    nc.vector.tensor_tensor(out=mt[:], in0=mt[:], in1=ltf[:],
                                            op=mybir.AluOpType.add)
                    nc.sync.dma_start(
                        out=M3[:, sl, :],
                        in_=mt[:].rearrange("p (n d) -> p n d", d=d))

                if ell < n_layers - 1:
                    nc.gpsimd.collective_compute(
                        "AllGather", mybir.AluOpType.bypass,
                        replica_groups=rg,
                        ins=[L[0:rcore, :].opt()],
                        outs=[tabs[ell][:, :].opt()])

            # ---- final loss phase (identical on every core) ----
            nc.gpsimd.collective_compute(
                "AllGather", mybir.AluOpType.bypass, replica_groups=rg,
                ins=[M[0:rcore, :].opt()], outs=[Mfull[:, :].opt()])
            ug = fpool.tile([P, bch * d], F32)
            vg = fpool.tile([P, bch * d], F32)
            nc.gpsimd.indirect_dma_start(
                out=ug[:], out_offset=None, in_=Mfull[:, :],
                in_offset=bass.IndirectOffsetOnAxis(ap=urow_sb[:, :], axis=0))
            nc.gpsimd.indirect_dma_start(
                out=vg[:], out_offset=None, in_=Mfull[:, :],
                in_offset=bass.IndirectOffsetOnAxis(ap=vrow_sb[:, :], axis=0))
            prod = fpool.tile([P, bch * d], F32)
            nc.vector.tensor_tensor(out=prod[:], in0=ug[:], in1=vg[:],
                                    op=mybir.AluOpType.mult)
            gam = fpool.tile([P, bch], F32)
            nc.vector.tensor_reduce(
                out=gam[:], in_=prod[:].rearrange("p (b d) -> p b d", d=d),
                axis=mybir.AxisListType.X, op=mybir.AluOpType.add)
            # logits = gamma / (n_layers+1)^2  (mean pooling of u and v)
            sc = 1.0 / float((n_layers + 1) ** 2)
            relu = fpool.tile([P, bch], F32)
            nc.scalar.activation(out=relu[:], in_=gam[:], func=AT.Relu,
                                 scale=sc)
            absg = fpool.tile([P, bch], F32)
            nc.scalar.activation(out=absg[:], in_=gam[:], func=AT.Abs,
                                 scale=sc)
            expn = fpool.tile([P, bch], F32)
            nc.scalar.activation(out=expn[:], in_=absg[:], func=AT.Exp,
                                 scale=-1.0)
            sp = fpool.tile([P, bch], F32)
            nc.scalar.activation(out=sp[:], in_=expn[:], func=AT.Ln,
                                 bias=1.0)
            gy = fpool.tile([P, bch], F32)
            nc.vector.scalar_tensor_tensor(
                out=gy[:], in0=gam[:], scalar=sc, in1=lab_sb[:],
                op0=mybir.AluOpType.mult, op1=mybir.AluOpType.mult)
            e1 = fpool.tile([P, bch], F32)
            nc.vector.tensor_tensor(out=e1[:], in0=relu[:], in1=gy[:],
                                    op=mybir.AluOpType.subtract)
            red = fpool.tile([P, 1], F32)
            nc.vector.scalar_tensor_tensor(
                out=e1[:], in0=e1[:], scalar=0.0, in1=sp[:],
                op0=mybir.AluOpType.add, op1=mybir.AluOpType.add,
                accum_out=red[:])
            ps1 = ppool.tile([1, 1], F32, tag="ps1", bufs=1)
            nc.tensor.matmul(out=ps1[:], lhsT=red[:], rhs=ones_sb[:],
                             start=True, stop=True)
            lsb = fpool.tile([1, 1], F32)
            nc.scalar.mul(out=lsb[:], in_=ps1[:], mul=1.0 / cfg["batch"])
            nc.sync.dma_start(out=loss.ap(), in_=lsb[:])

    nc.finalize()
    return nc


# ============================ entry point ===============================

_CACHE = {}


def kernel(users, items, labels, edge_row, edge_col, edge_val,
           user_emb, item_emb):
    cfg = REAL
    users = np.asarray(users)
    items = np.asarray(items)
    labels = np.asarray(labels)
    edge_row = np.asarray(edge_row)
    edge_col = np.asarray(edge_col)
    edge_val = np.asarray(edge_val)
    user_emb = np.asarray(user_emb)
    item_emb = np.asarray(item_emb)

    in_maps, NG = preprocess(users, items, labels, edge_row, edge_col,
                             edge_val, user_emb, item_emb, cfg)
    key = NG
    if key not in _CACHE:
        _CACHE[key] = build_program(cfg, NG)
    nc = _CACHE[key]
    res = bass_utils.run_bass_kernel_spmd(nc, in_maps,
                                          core_ids=list(range(W)))
    global LAST_RESULT
    LAST_RESULT = res
    out = res.results[0]["loss"]
    return np.float32(out.reshape(())).reshape(())



# revision 50
# speedup vs baseline: 1.3010x; 1.3010x over previous
"""LightGCN (3-layer propagation + BCE loss) on 8 Trainium2 NeuronCores.

v2 strategy (row-sharded graph parallelism, fp8 datapath):
  - Rows are sharded across 8 cores (18816 rows/core).  Edges are sorted
    by destination row; consecutive rows are greedily packed into groups
    of <=64 rows and <=896 edges.  Each group's edges fill C=7 chunks of
    128 slots (one slot per partition).
  - The per-chunk selection matrices sel_val[slot, local_row] = 16*edge_val
    (one-hot times value, x16 so fp8 stays in normal range) are built ON
    THE HOST in fp8 and stay resident in SBUF for all 3 layers -- the
    device does no selection-building or value-scaling work at all.
  - Per layer: mega-batched indirect gathers (128 chunks = 16K rows per
    DMA) pull fp8 table rows; each chunk does one 128x64 matmul
    (lhsT=sel_val chunk, rhs=gathered rows) accumulating segment sums for
    its group into a PSUM quad (4 groups of 64 rows share one [128,128]
    psum tile); ACT copies quads into fp8 staging; batched indirect
    scatters (24 row-pairs each) write the new table shard to DRAM.
  - Tables are stored scaled by 16^layer; the mean-pool accumulator M
    (fp32, SBUF-resident) adds each layer shard times 16^-layer.
  - An AllGather per layer rebuilds the full fp8 table; a final AllGather
    of the fp32 mean table feeds the BCE-with-logits loss, computed
    redundantly on every core.
"""

import sys

sys.path.insert(0, "/opt/trn_rl_repo")

import numpy as np

import concourse.bacc as bacc
import concourse.bass as bass
import concourse.mybir as mybir
import concourse.tile as tile
from concourse import bass_utils

F8 = mybir.dt.float8e4
F16 = mybir.dt.float16
F32 = mybir.dt.float32
I32 = mybir.dt.int32
NP_F8 = mybir.dt.np(F8)

REAL = dict(
    n_users=100001,
    n_total=150001,
    d=64,
    n_layers=3,
    batch=8192,
    n_blocks=147,   # rcore = 147*128 = 18816 rows per core
    RW=64,          # max rows per group
    C=7,            # 128-slot chunks per group (<=896 edges)
    NMEGA=8,        # mega-gathers per layer
    SBP=38,         # row-pairs per scatter batch (must be even)
    NUBC=27,        # compact final-exchange block columns (128*27 slots)
)
W = 8
P = 128
SEL_SCALE = 16.0  # per-layer table scale factor
UPAD = 64    # zero tail per compact block (keeps contiguous over-reads of
             # degraded indirect-DMA backends in-bounds and finite)


# ======================= host-side preprocessing ========================


def _greedy_groups(counts, rcore, RW, S):
    """Split rows [0, rcore) into consecutive groups with <=RW rows and
    <=S edges.  Returns (r0s, nrs) arrays."""
    cum = np.concatenate([[0], np.cumsum(counts)])
    r0s, nrs = [], []
    r = 0
    while r < rcore:
        r1_edge = int(np.searchsorted(cum, cum[r] + S, side="right")) - 1
        r1 = min(r + RW, r1_edge, rcore)
        assert r1 > r, "row degree exceeds group capacity"
        r0s.append(r)
        nrs.append(r1 - r)
        r = r1
    return np.asarray(r0s, np.int64), np.asarray(nrs, np.int64), cum


def preprocess(users, items, labels, edge_row, edge_col, edge_val,
               user_emb, item_emb, cfg):
    d = cfg["d"]
    C = cfg["C"]
    RW = cfg["RW"]
    S = C * P
    rcore = cfg["n_blocks"] * P
    n_pad = rcore * W
    nblk = cfg["n_blocks"]

    order = np.argsort(edge_row, kind="stable")
    rows = edge_row[order]
    cols = edge_col[order]
    vals = edge_val[order].astype(np.float64)

    bounds = np.searchsorted(rows, np.arange(W + 1) * rcore)
    per_core = []
    for k in range(W):
        s, e = bounds[k], bounds[k + 1]
        rl = (rows[s:e] - k * rcore).astype(np.int64)
        counts = np.bincount(rl, minlength=rcore)
        r0s, nrs, cum = _greedy_groups(counts, rcore, RW, S)
        per_core.append((rl, cols[s:e], vals[s:e], r0s, nrs, cum))

    NGRP = max(len(pc[3]) for pc in per_core)
    NGRP += NGRP % 2  # even
    NCH = NGRP * C
    NPAIR = NGRP // 2

    # full fp8 table in the padded rank-block layout: each core's block is
    # lrows = rcore+128 rows (real shard + zeroed dummy tail), matching the
    # per-layer AllGather of L (which includes the zeroed tail)
    lrows = rcore + P
    x0 = np.zeros((n_pad, d), np.float32)
    x0[:user_emb.shape[0]] = user_emb
    x0[user_emb.shape[0]:user_emb.shape[0] + item_emb.shape[0]] = item_emb
    table0 = np.zeros((W * lrows, d), NP_F8)
    for k in range(W):
        table0[k * lrows:k * lrows + rcore] = \
            x0[k * rcore:(k + 1) * rcore].astype(NP_F8)

    bch = cfg["batch"] // P
    lab = labels.astype(np.float32).reshape(P, bch)
    ones = np.ones((P, 1), np.float32)

    # --- compact final-phase exchange tables ---
    # each core gathers the batch rows it owns into a compact block; one
    # AllGather of the blocks replaces the full-mean-table AllGather.
    B = cfg["batch"]
    rows16 = np.concatenate([users.astype(np.int64),
                             cfg["n_users"] + items.astype(np.int64)])
    owner = rows16 // rcore
    NUBC = cfg["NUBC"]  # compact block columns (block = 128*NUBC slots)
    nub = P * NUBC
    slot = np.zeros(2 * B, np.int64)
    comp_idxs = []
    for k in range(W):
        mine = np.nonzero(owner == k)[0]
        assert len(mine) <= nub, f"core {k} owns {len(mine)} > {nub}"
        slot[mine] = np.arange(len(mine))
        ci = np.zeros(P * NUBC, np.int32)
        ci[:len(mine)] = (rows16[mine] - k * rcore).astype(np.int32)
        # slot s -> (partition s // NUBC, column s % NUBC)
        comp_idxs.append(np.ascontiguousarray(
            ci.reshape(P, NUBC)))
    gslot = owner * (nub + UPAD) + slot
    reasm_u = gslot[:B].astype(np.int32).reshape(P, bch)
    reasm_v = gslot[B:].astype(np.int32).reshape(P, bch)

    in_maps = []
    for k in range(W):
        rl, ck, vk, r0s, nrs, cum = per_core[k]
        ngk = len(r0s)
        nE = len(rl)

        # pad group tables
        r0p = np.full(NGRP, rcore, np.int64)
        nrp = np.zeros(NGRP, np.int64)
        r0p[:ngk] = r0s
        nrp[:ngk] = nrs

        e0s = cum[r0s]
        # edge -> (partition, chunk) placement
        gi = np.repeat(np.arange(ngk), (cum[r0s + nrs] - e0s).astype(np.int64))
        sl = np.arange(nE) - np.repeat(e0s, (cum[r0s + nrs] - e0s).astype(np.int64))
        cc = sl >> 7
        pp = sl & 127
        tt = gi * C + cc
        assert cc.max(initial=0) < C

        colidx = np.zeros((P, NCH), np.int32)
        ck64 = ck.astype(np.int64)
        colidx[pp, tt] = (ck64 // rcore * lrows + ck64 % rcore).astype(np.int32)
        selval = np.zeros((P, NCH * RW), np.float32)
        off = rl - np.repeat(r0s, (cum[r0s + nrs] - e0s).astype(np.int64))
        selval[pp, tt * RW + off] = SEL_SCALE * vk

        # pad rows of each group duplicate the group's row 0 so scatters
        # stay idempotent (same value written to the same address)
        jj, rr = np.nonzero(np.arange(RW)[None, :] >= np.maximum(nrp, 1)[:, None])
        if len(jj):
            for c in range(C):
                tcol = (jj * C + c) * RW
                selval[:, tcol + rr] = selval[:, tcol]

        # scatter targets (absolute local rows; pads -> group row0 / dummy)
        tgt = np.zeros((P, NPAIR), np.int32)
        pa = np.arange(64)
        jA = 2 * np.arange(NPAIR)
        jB = jA + 1
        offA = np.where(pa[:, None] < nrp[jA][None, :], pa[:, None], 0)
        offB = np.where(pa[:, None] < nrp[jB][None, :], pa[:, None], 0)
        tgt[:64, :] = (r0p[jA][None, :] + offA).astype(np.int32)
        tgt[64:, :] = (r0p[jB][None, :] + offB).astype(np.int32)

        # fp32 layer-0 contribution of this core's compact rows
        ci = comp_idxs[k].reshape(-1)
        t0c = np.ascontiguousarray(
            x0[k * rcore + ci.astype(np.int64)].reshape(P, NUBC * d))

        in_maps.append({
            "table0": table0,
            "t0c": t0c,
            "colidx": colidx,
            "selval": np.ascontiguousarray(selval.astype(NP_F8)),
            "tgt": tgt,
            "compidx": comp_idxs[k],
            "reasmu": reasm_u,
            "reasmv": reasm_v,
            "labels": lab,
            "ones": ones,
        })
    return in_maps, NGRP


# =========================== device program =============================


def build_program(cfg, NGRP):
    d = cfg["d"]
    C = cfg["C"]
    RW = cfg["RW"]
    SBP = cfg["SBP"]
    nblk = cfg["n_blocks"]
    rcore = nblk * P
    n_pad = rcore * W
    lrows = rcore + P
    bch = cfg["batch"] // P
    n_layers = cfg["n_layers"]
    NCH = NGRP * C
    NPAIR = NGRP // 2
    # MB <= 128 so a degraded backend's contiguous over-read stays within
    # the 128-row zero pad trailing each rank block
    NMEGA = max(cfg["NMEGA"], (NCH + 127) // 128)
    MB = (NCH + NMEGA - 1) // NMEGA

    nc = bacc.Bacc("TRN2", target_bir_lowering=False, debug=False,
                   enable_asserts=False, num_devices=W)

    NUBC_ = cfg["NUBC"]
    table0 = nc.dram_tensor("table0", [W * lrows, d], F8,
                            kind="ExternalInput")
    t0c = nc.dram_tensor("t0c", [P, NUBC_ * d], F32, kind="ExternalInput")
    colidx = nc.dram_tensor("colidx", [P, NCH], I32, kind="ExternalInput")
    selval = nc.dram_tensor("selval", [P, NCH * RW], F8, kind="ExternalInput")
    tgt = nc.dram_tensor("tgt", [P, NPAIR], I32, kind="ExternalInput")
    NUBC = cfg["NUBC"]
    nub = P * NUBC
    compidx = nc.dram_tensor("compidx", [P, NUBC], I32, kind="ExternalInput")
    reasmu = nc.dram_tensor("reasmu", [P, bch], I32, kind="ExternalInput")
    reasmv = nc.dram_tensor("reasmv", [P, bch], I32, kind="ExternalInput")
    labels = nc.dram_tensor("labels", [P, bch], F32, kind="ExternalInput")
    ones = nc.dram_tensor("ones", [P, 1], F32, kind="ExternalInput")
    loss = nc.dram_tensor("loss", [1, 1], F32, kind="ExternalOutput")
    dbg = cfg.get("debug_dump")
    if dbg:
        dbgL = nc.dram_tensor("dbgL", [P, nblk * d], F8, kind="ExternalOutput")
        dbgA = nc.dram_tensor("dbgA", [P, NUBC_ * d], F32,
                              kind="ExternalOutput")
        dbgG = nc.dram_tensor("dbgG", [P, MB * d], F8, kind="ExternalOutput")

    rg = [list(range(W))]
    AT = mybir.ActivationFunctionType
    NSEL = 4  # selval load split for startup overlap

    with tile.TileContext(nc) as tc:
        with (
            tc.tile_pool(name="dram", bufs=1, space="DRAM") as dpool,
            tc.tile_pool(name="mpool", bufs=1) as mpool,
            tc.tile_pool(name="small", bufs=1) as spool_c,
        ):
            Ls = [dpool.tile([lrows, d], F8, name=f"L{i}") for i in range(2)]
            tabs = [dpool.tile([W * lrows, d], F8, name=f"tab{i}",
                               addr_space="Shared")
                    for i in range(n_layers - 1)]
            blk = dpool.tile([nub + UPAD, d], F8)
            Uall = dpool.tile([W * (nub + UPAD), d], F8, addr_space="Shared")

            acc = mpool.tile([P, NUBC * d], F32)
            ru_sb = spool_c.tile([P, bch], I32)
            rv_sb = spool_c.tile([P, bch], I32)
            ci_sb = spool_c.tile([P, NUBC], I32)
            lab_sb = spool_c.tile([P, bch], F32)
            ones_sb = spool_c.tile([P, 1], F32)
            for sb, dr in ((ru_sb, reasmu), (rv_sb, reasmv),
                           (ci_sb, compidx), (lab_sb, labels),
                           (ones_sb, ones)):
                nc.sync.dma_start(out=sb[:], in_=dr.ap())
            nc.sync.dma_start(out=acc[:], in_=t0c.ap())

            with (
                tc.tile_pool(name="const", bufs=1) as cpool,
                tc.tile_pool(name="gt", bufs=2) as wpool,
                tc.tile_pool(name="stage", bufs=2) as stpool,
                tc.tile_pool(name="psum", bufs=4, space="PSUM") as ppool,
            ):
                colidx_sb = cpool.tile([P, NCH], I32)
                tgt_sb = cpool.tile([P, NPAIR], I32)
                selval_sb = cpool.tile([P, NCH * RW], F8)
                nc.sync.dma_start(out=colidx_sb[:], in_=colidx.ap())
                nc.sync.dma_start(out=tgt_sb[:], in_=tgt.ap())
                # zero-init L buffers (their zeroed dummy tails ride along
                # in the AllGather) and the compact-block pad: keeps every
                # byte a degraded backend can over-read finite
                nlb = lrows // P
                zt = cpool.tile([P, nlb * d], F8)
                nc.scalar.memzero(zt[:])
                for Lx in Ls:
                    nc.sync.dma_start(
                        out=Lx[:, :].rearrange("(p n) d -> p n d", p=P),
                        in_=zt[:].rearrange("p (n d) -> p n d", d=d))
                nc.sync.dma_start(
                    out=blk[nub:nub + UPAD, :].rearrange(
                        "(p n) d -> p n d", p=UPAD),
                    in_=zt[0:UPAD, 0:d].rearrange("p (n d) -> p n d", d=d))
                selq = (NCH + NSEL - 1) // NSEL
                for i in range(NSEL):
                    a, b = i * selq, min((i + 1) * selq, NCH)
                    nc.sync.dma_start(
                        out=selval_sb[:, a * RW:b * RW],
                        in_=selval.ap()[:, a * RW:b * RW])

                for ell in range(n_layers):
                    tab_in = table0.ap() if ell == 0 else tabs[ell - 1][:, :]
                    Ld = Ls[ell % 2]

                    gts = {}

                    def emit_gather(m):
                        a = m * MB
                        b = min(a + MB, NCH)
                        g = wpool.tile([P, MB * d], F8, tag="gt")
                        nc.gpsimd.indirect_dma_start(
                            out=g[:, :(b - a) * d], out_offset=None,
                            in_=tab_in,
                            in_offset=bass.IndirectOffsetOnAxis(
                                ap=colidx_sb[:, a:b], axis=0))
                        gts[m] = g

                    emit_gather(0)
                    if dbg and ell == 0:
                        nc.sync.dma_start(out=dbgG.ap(), in_=gts[0][:, :])
                    ps = None
                    stage = None
                    nst = 0
                    for m in range(NMEGA):
                        if m + 1 < NMEGA:
                            emit_gather(m + 1)
                        a = m * MB
                        b = min(a + MB, NCH)
                        g = gts[m]
                        for t in range(a, b):
                            j, c = divmod(t, C)
                            h = j % 2
                            q = j // 2
                            cq = q % 2
                            if h == 0 and cq == 0 and c == 0:
                                ps = ppool.tile([P, 2 * d], F32, tag="ps")
                            nc.tensor.matmul(
                                out=ps[h * d:(h + 1) * d, cq * d:(cq + 1) * d],
                                lhsT=selval_sb[:, t * RW:(t + 1) * RW],
                                rhs=g[:, (t - a) * d:(t - a + 1) * d],
                                start=(c == 0), stop=(c == C - 1))
                            # quad (or tail) complete -> stage it
                            done_quad = (c == C - 1 and h == 1 and
                                         (cq == 1 or q == NPAIR - 1))
                            if done_quad:
                                q0 = q - cq  # first pair of this psum tile
                                if q0 % SBP == 0:
                                    stage = stpool.tile([P, SBP * d], F8,
                                                        tag="st")
                                    nst = 0
                                w = (cq + 1) * d
                                nc.scalar.copy(
                                    out=stage[:, (q0 % SBP) * d:
                                              (q0 % SBP) * d + w],
                                    in_=ps[:, :w])
                                nst += cq + 1
                                qlast = q0 + cq
                                if (qlast + 1) % SBP == 0 or qlast == NPAIR - 1:
                                    b0 = (q0 // SBP) * SBP
                                    nc.gpsimd.indirect_dma_start(
                                        out=Ld[:, :],
                                        out_offset=bass.IndirectOffsetOnAxis(
                                            ap=tgt_sb[:, b0:b0 + nst], axis=0),
                                        in_=stage[:, :nst * d],
                                        in_offset=None)
                                    nst = 0

                    # mean-pool: accumulate this layer's compact rows
                    cgl = stpool.tile([P, NUBC * d], F8, tag="cgl")
                    nc.gpsimd.indirect_dma_start(
                        out=cgl[:], out_offset=None, in_=Ld[:, :],
                        in_offset=bass.IndirectOffsetOnAxis(ap=ci_sb[:, :],
                                                            axis=0))
                    nc.vector.scalar_tensor_tensor(
                        out=acc[:], in0=cgl[:],
                        scalar=float(SEL_SCALE ** (-(ell + 1))),
                        in1=acc[:], op0=mybir.AluOpType.mult,
                        op1=mybir.AluOpType.add)

                    if ell < n_layers - 1:
                        nc.gpsimd.collective_compute(
                            "AllGather", mybir.AluOpType.bypass,
                            replica_groups=rg,
                            ins=[Ld[:, :].opt()],
                            outs=[tabs[ell][:, :].opt()])

            # ---- final loss phase: compact exchange of batch rows ----
            if dbg:
                nc.sync.dma_start(
                    out=dbgL.ap().rearrange("p (n d) -> p n d", d=d),
                    in_=Ls[(n_layers - 1) % 2][0:rcore, :].rearrange(
                        "(p n) d -> p n d", p=P))
                nc.sync.dma_start(out=dbgA.ap(), in_=acc[:])
            with tc.tile_pool(name="fin", bufs=1) as fpool:
                cg8 = fpool.tile([P, NUBC * d], F8)
                nc.scalar.copy(out=cg8[:], in_=acc[:])
                nc.sync.dma_start(
                    out=blk[0:nub, :].rearrange("(p c) d -> p c d", p=P),
                    in_=cg8[:].rearrange("p (c d) -> p c d", d=d))
                nc.gpsimd.collective_compute(
                    "AllGather", mybir.AluOpType.bypass, replica_groups=rg,
                    ins=[blk[:, :].opt()], outs=[Uall[:, :].opt()])
                ug = fpool.tile([P, bch * d], F8)
                vg = fpool.tile([P, bch * d], F8)
                nc.gpsimd.indirect_dma_start(
                    out=ug[:], out_offset=None, in_=Uall[:, :],
                    in_offset=bass.IndirectOffsetOnAxis(ap=ru_sb[:, :],
                                                        axis=0))
                nc.gpsimd.indirect_dma_start(
                    out=vg[:], out_offset=None, in_=Uall[:, :],
                    in_offset=bass.IndirectOffsetOnAxis(ap=rv_sb[:, :],
                                                        axis=0))
                prod = fpool.tile([P, bch * d], F32)
                nc.vector.tensor_tensor(out=prod[:], in0=ug[:], in1=vg[:],
                                        op=mybir.AluOpType.mult)
                gam = fpool.tile([P, bch], F32)
                nc.vector.tensor_reduce(
                    out=gam[:], in_=prod[:].rearrange("p (b d) -> p b d", d=d),
                    axis=mybir.AxisListType.X, op=mybir.AluOpType.add)
                sc = 1.0 / float((n_layers + 1) ** 2)
                relu = fpool.tile([P, bch], F32)
                nc.scalar.activation(out=relu[:], in_=gam[:], func=AT.Relu,
                                     scale=sc)
                absg = fpool.tile([P, bch], F32)
                nc.scalar.activation(out=absg[:], in_=gam[:], func=AT.Abs,
                                     scale=sc)
                expn = fpool.tile([P, bch], F32)
                nc.scalar.activation(out=expn[:], in_=absg[:], func=AT.Exp,
                                     scale=-1.0)
                sp = fpool.tile([P, bch], F32)
                nc.scalar.activation(out=sp[:], in_=expn[:], func=AT.Ln,
                                     bias=1.0)
                gy = fpool.tile([P, bch], F32)
                nc.vector.scalar_tensor_tensor(
                    out=gy[:], in0=gam[:], scalar=sc, in1=lab_sb[:],
                    op0=mybir.AluOpType.mult, op1=mybir.AluOpType.mult)
                e1 = fpool.tile([P, bch], F32)
                nc.vector.tensor_tensor(out=e1[:], in0=relu[:], in1=gy[:],
                                        op=mybir.AluOpType.subtract)
                red = fpool.tile([P, 1], F32)
                nc.vector.scalar_tensor_tensor(
                    out=e1[:], in0=e1[:], scalar=0.0, in1=sp[:],
                    op0=mybir.AluOpType.add, op1=mybir.AluOpType.add,
                    accum_out=red[:])
                with tc.tile_pool(name="ps1", bufs=1, space="PSUM") as p1:
                    ps1 = p1.tile([1, 1], F32)
                    nc.tensor.matmul(out=ps1[:], lhsT=red[:], rhs=ones_sb[:],
                                     start=True, stop=True)
                    lsb = fpool.tile([1, 1], F32)
                    nc.scalar.mul(out=lsb[:], in_=ps1[:],
                                  mul=1.0 / cfg["batch"])
                    nc.sync.dma_start(out=loss.ap(), in_=lsb[:])

    nc.finalize()
    return nc


# ============================ entry point ===============================

_CACHE = {}


def kernel(users, items, labels, edge_row, edge_col, edge_val,
           user_emb, item_emb):
    cfg = REAL
    users = np.asarray(users)
    items = np.asarray(items)
    labels = np.asarray(labels)
    edge_row = np.asarray(edge_row)
    edge_col = np.asarray(edge_col)
    edge_val = np.asarray(edge_val)
    user_emb = np.asarray(user_emb)
    item_emb = np.asarray(item_emb)

    in_maps, NGRP = preprocess(users, items, labels, edge_row, edge_col,
                               edge_val, user_emb, item_emb, cfg)
    key = NGRP
    if key not in _CACHE:
        _CACHE[key] = build_program(cfg, NGRP)
    nc = _CACHE[key]
    res = bass_utils.run_bass_kernel_spmd(nc, in_maps,
                                          core_ids=list(range(W)))
    global LAST_RESULT
    LAST_RESULT = res
    out = res.results[0]["loss"]
    return np.float32(out.reshape(())).reshape(())
